# revision 1
# baseline (speedup 1.0000x reference)
"""Trainium2 Bass kernel for nn_LowPass (order-2 Butterworth filtfilt).

Algorithm
---------
filtfilt = causal IIR pass + anticausal IIR pass. The biquad's impulse
response h decays like r^n with r ~= 0.9726, negligible after NT=640
taps, so each IIR pass is computed exactly (to fp32 noise) as a 640-tap
FIR. In a chunked layout (time n = 128*c + p: partition p, chunk col c)
each pass is 5 banded 128x128 matmuls per chunk column on the PE:

    stage1 (causal):     y1[:, c] = sum_j G1_j^T @ x[:, c-j]
    mask:                y1 zeroed on [row_end, next_row_start)  (matches
                         the reference's zero-state backward truncation)
    stage2 (anticausal): y2[:, c] = sum_j G2_j^T @ y1[:, c+j]

with G1_j[p', p] = h[128j + p - p'], G2_j = G1_j^T. Both stages are fused
per 512-column block in SBUF/PSUM; HBM traffic is read-x + write-y only.

The reference's per-row max-abs scaling is skipped: the whole pipeline is
linear in x, so scale * filt(x/scale) == filt(x) up to fp32 rounding.

Rows are data-parallel: 40 rows per core on 8 cores. Each core's rows are
concatenated (odd-extended, with >=NT zero gap) into one stream, chopped
into 123 blocks of 512 chunk-columns with 4-column halos on the host.
"""
import math
import numpy as np

import concourse.bass as bass
import concourse.mybir as mybir
from concourse.tile import TileContext
from concourse.vector_clock import ScopedClock
from concourse import bass_utils

# ---------------------------------------------------------------------------
# Compat patches: this walrus build supports only one sync-wait command per
# TPB_CTRL instruction, so split Tile's exit-drain waits and use the
# sem-only all-engine barrier (no eq-wait drains).
# ---------------------------------------------------------------------------
def _patched_meb(self, engines):
    for inst in self._sem_only_all_engine_barrier_insts(f"aeb{self.next_id()}"):
        self.engines[inst.engine].add_instruction(inst)


def _patched_dab(self, tick_clock, wait_clock):
    drain_inst = self.nc.sync.drain()
    wait_clock.add_sem_waits(
        drain_inst.ins, ScopedClock({None: tick_clock.global_clock})
    )
    si = drain_inst.ins.sync_info
    if si is not None and si.on_wait and len(si.on_wait) > 1:
        waits = list(si.on_wait)
        si.on_wait = waits[:1]
        for w in waits[1:]:
            d2 = self.nc.sync.drain()
            d2.ins.sync_info = mybir.SyncInfo(on_wait=[w], on_update=[])
    self.nc.all_engine_barrier()
    popped = self.nc._tile_sem_poison_stack.pop()
    assert popped is self._sem_poison
    self.nc.clear_and_free_semaphores(list(self.sems.allocated().values()))
    self.nc.all_engine_barrier()


bass.Bass.multi_engine_barrier = _patched_meb
TileContext._drain_and_barrier = _patched_dab


def _split_multi_waits(nc):
    """Walrus here allows one sync-wait command per engine instruction:
    hoist extra waits onto InstNoOp carriers inserted just before."""
    import copy as _copy
    nop_template = None
    counter = [0]

    def _mk_nop(engine, wait):
        nop = _copy.replace(nop_template, name=f"I-waitsplit-{counter[0]}")
        counter[0] += 1
        nop.engine = engine
        nop.sync_info = mybir.SyncInfo(on_wait=[wait], on_update=[])
        return nop

    m = nc.m
    for fn in m.functions:
        for blk in fn.blocks:
            need = False
            for inst in blk.instructions:
                si = inst.sync_info
                if (si is not None and si.on_wait and len(si.on_wait) > 1
                       ):
                    need = True
                    break
            if not need:
                continue
            insts = []
            for inst in blk.instructions:
                si = inst.sync_info
                if (si is not None and si.on_wait and len(si.on_wait) > 1
                       ):
                    if nop_template is None:
                        import bass_rust
                        nop_template = bass_rust.InstNoOp(name="I-waitsplit-t")
                    ws = list(si.on_wait)
                    for w in ws[:-1]:
                        insts.append(_mk_nop(inst.engine, w))
                    si.on_wait = ws[-1:]
                insts.append(inst)
            blk.instructions[:] = []
            for i in insts:
                blk.instructions.append(i)

# ---------------------------------------------------------------------------
# Layout constants (hardcoded for x of shape (320, 200000) on 8 cores)
# ---------------------------------------------------------------------------
T = 200000
PADLEN = 9
TXE = T + 2 * PADLEN          # 200018 odd-extended row length
NT = 640                      # truncated impulse response taps
J = 5                         # bands per stage (NT/128)
P = 128
ROW_CHUNKS = 1568             # chunk columns per row slot
S = ROW_CHUNKS * P            # 200704 row stride in stream (gap = 686 >= NT)
P0 = 768                      # stream left pad (6 chunks)
BLK = 512                     # output chunk columns per block (one PSUM bank)
Y1_COLS = 520                 # stage-1 columns computed per block
SLAB = 524                    # input slab columns per block [c0-4, c0+520)
HALO_L = 4
NCORES = 8
ROWS_PER_CORE = 40
NBLOCKS = math.ceil((P0 + ROWS_PER_CORE * S) / (BLK * P))   # 123
NCHUNK = NBLOCKS * BLK        # 62976 chunk columns per core
USE_F32R = False              # plain fp32 matmuls (4 cyc/row, exact)


def _impulse_response(b, a, nt):
    b = np.asarray(b, np.float64)
    a = np.asarray(a, np.float64)
    b = b / a[0]
    a = a / a[0]
    h = np.zeros(nt, np.float64)
    for n in range(nt):
        acc = b[n] if n < len(b) else 0.0
        for k in range(1, len(a)):
            if n - k >= 0:
                acc -= a[k] * h[n - k]
        h[n] = acc
    return h


def _band_matrices(h):
    idx = np.arange(P)
    G = np.zeros((2 * J, P, P), np.float64)
    for j in range(J):
        k = 128 * j + idx[None, :] - idx[:, None]     # [p', p]: k = 128j + p - p'
        valid = (k >= 0) & (k < NT)
        G[j][valid] = h[np.clip(k, 0, NT - 1)][valid]
    for j in range(J):
        G[J + j] = G[j].T
    return G.astype(np.float32)


def _mask_schedule():
    """Per block k: list of (col_lo, col_hi, p_lo, p_hi) regions of the
    block-local y1 tile to zero. Row r's xe ends at absolute position
    P0 + r*S + TXE; y1 must be zero from there to the next row start."""
    sched = {k: [] for k in range(NBLOCKS)}
    e_rem = (P0 + TXE) % P                      # 82
    for r in range(ROWS_PER_CORE):
        c_e = (P0 + r * S + TXE) // P           # partial chunk
        regions = [(c_e, c_e + 1, e_rem, P)] + [(c_e + 1, c_e + 6, 0, P)]
        for (clo, chi, plo, phi) in regions:
            for k in range(NBLOCKS):
                b_lo, b_hi = k * BLK, k * BLK + Y1_COLS
                lo, hi = max(clo, b_lo), min(chi, b_hi)
                if lo < hi:
                    sched[k].append((lo - k * BLK, hi - k * BLK, plo, phi))
    return sched


def _build(reps=1):
    nc = bass.Bass()
    g = nc.dram_tensor("g", [2 * J * P, P], mybir.dt.float32, kind="ExternalInput")
    mv = nc.dram_tensor("mv", [P, 1], mybir.dt.float32, kind="ExternalInput")
    xin = nc.dram_tensor("xin", [NBLOCKS, P, SLAB], mybir.dt.float32,
                         kind="ExternalInput")
    yout = nc.dram_tensor("yout", [NBLOCKS, P, BLK], mybir.dt.float32,
                          kind="ExternalOutput")
    sched = _mask_schedule()
    with TileContext(nc) as tc:
        with (
            tc.tile_pool(name="gp", bufs=1) as gp,
            tc.tile_pool(name="xp", bufs=4) as xp,
            tc.tile_pool(name="y1p", bufs=3) as y1p,
            tc.tile_pool(name="yp", bufs=4) as yp,
            tc.tile_pool(name="pp1", bufs=2, space="PSUM") as pp1,
            tc.tile_pool(name="pp1b", bufs=2, space="PSUM") as pp1b,
            tc.tile_pool(name="pp2", bufs=2, space="PSUM") as pp2,
        ):
            gt = gp.tile([P, 2 * J * P], mybir.dt.float32)
            mvt = gp.tile([P, 1], mybir.dt.float32)
            nc.sync.dma_start(mvt[:], mv[:])
            for j in range(2 * J):
                nc.sync.dma_start(gt[:, j * P:(j + 1) * P], g[j * P:(j + 1) * P, :])

            def band1(j):
                return gt[:, j * P:(j + 1) * P]

            def band2(j):
                return gt[:, (J + j) * P:(J + j + 1) * P]

            for rep in range(reps):
                for k in range(NBLOCKS):
                    xt = xp.tile([P, SLAB], mybir.dt.float32)
                    nc.sync.dma_start(xt[:], xin[k])
                    # stage 1: y1 cols [0, 512) and sliver [512, 520)
                    ps1 = pp1.tile([P, BLK], mybir.dt.float32)
                    for j in range(J):
                        off = HALO_L - j
                        nc.tensor.matmul(ps1[:], band1(j), xt[:, off:off + BLK],
                                         start=(j == 0), stop=(j == J - 1))
                    ps1b = pp1b.tile([P, 8], mybir.dt.float32)
                    for j in range(J):
                        off = BLK + HALO_L - j
                        nc.tensor.matmul(ps1b[:], band1(j), xt[:, off:off + 8],
                                         start=(j == 0), stop=(j == J - 1))
                    y1t = y1p.tile([P, Y1_COLS], mybir.dt.float32)
                    nc.scalar.copy(y1t[:, :BLK], ps1[:])
                    nc.scalar.copy(y1t[:, BLK:Y1_COLS], ps1b[:])
                    # mask: zero y1 on [row_end, next_row_start)
                    for (clo, chi, plo, phi) in sched[k]:
                        if plo == 0:
                            nc.gpsimd.memset(y1t[plo:phi, clo:chi], 0.0)
                        else:
                            nc.vector.tensor_scalar_mul(
                                y1t[:, clo:chi], y1t[:, clo:chi], mvt[:])
                    # stage 2
                    ps2 = pp2.tile([P, BLK], mybir.dt.float32)
                    for j in range(J):
                        nc.tensor.matmul(ps2[:], band2(j), y1t[:, j:j + BLK],
                                         start=(j == 0), stop=(j == J - 1))
                    yt = yp.tile([P, BLK], mybir.dt.float32)
                    nc.vector.tensor_copy(yt[:], ps2[:])
                    nc.sync.dma_start(yout[k], yt[:])
    return nc


def _odd_ext(x):
    left = 2.0 * x[:, :1] - x[:, 1:PADLEN + 1][:, ::-1]
    right = 2.0 * x[:, -1:] - x[:, -(PADLEN + 1):-1][:, ::-1]
    return np.concatenate([left, x, right], axis=1).astype(np.float32)


def _prep_core(xe_rows):
    """xe_rows: [ROWS_PER_CORE, TXE] f32 -> slabs [NBLOCKS, 128, SLAB]."""
    ncols = HALO_L + NCHUNK + (SLAB - HALO_L - BLK)
    pc = np.zeros((P, ncols), np.float32)
    stream = np.zeros(NCHUNK * P, np.float32)
    for r in range(xe_rows.shape[0]):
        o = P0 + r * S
        stream[o:o + TXE] = xe_rows[r]
    pc[:, HALO_L:HALO_L + NCHUNK] = stream.reshape(NCHUNK, P).T
    sk, sp, sc = pc.strides[1] * BLK, pc.strides[0], pc.strides[1]
    slabs = np.lib.stride_tricks.as_strided(
        pc, shape=(NBLOCKS, P, SLAB), strides=(sk, sp, sc))
    return np.ascontiguousarray(slabs)


def _gather_core(yblk):
    """yblk: [NBLOCKS, 128, BLK] -> rows [ROWS_PER_CORE, T]."""
    out_stream = yblk.transpose(0, 2, 1).reshape(-1)   # n = c*128 + p
    rows = np.empty((ROWS_PER_CORE, T), np.float32)
    for r in range(ROWS_PER_CORE):
        o = P0 + r * S + PADLEN
        rows[r] = out_stream[o:o + T]
    return rows


_NC_CACHE = {}


def _run(x, b, a, reps=1):
    x = np.asarray(x, np.float32)
    assert x.shape == (NCORES * ROWS_PER_CORE, T), x.shape
    h = _impulse_response(np.asarray(b), np.asarray(a), NT)
    g = _band_matrices(h).reshape(2 * J * P, P)
    xe = _odd_ext(x)
    in_maps = []
    for c in range(NCORES):
        slabs = _prep_core(xe[c * ROWS_PER_CORE:(c + 1) * ROWS_PER_CORE])
        mvec = (np.arange(P) < (P0 + TXE) % P).astype(np.float32).reshape(P, 1)
        in_maps.append({"g": g, "xin": slabs, "mv": mvec})
    if reps not in _NC_CACHE:
        nc = _build(reps)
        _split_multi_waits(nc)
        _NC_CACHE[reps] = nc
    import time
    t0 = time.perf_counter()
    res = bass_utils.run_bass_kernel_spmd(
        _NC_CACHE[reps], in_maps, core_ids=list(range(NCORES)))
    wall = time.perf_counter() - t0
    y = np.empty((NCORES * ROWS_PER_CORE, T), np.float32)
    for c in range(NCORES):
        y[c * ROWS_PER_CORE:(c + 1) * ROWS_PER_CORE] = _gather_core(
            res.results[c]["yout"])
    return y, wall


def kernel(x, b, a):
    y, _ = _run(x, b, a, reps=1)
    return y



# revision 2
# speedup vs baseline: 1.3304x; 1.3304x over previous
"""Trainium2 Bass kernel for nn_LowPass (order-2 Butterworth filtfilt).

filtfilt == causal IIR pass + anticausal IIR pass. The biquad's impulse
response decays like r^n with r ~= 0.9726, so each pass is computed as a
truncated-FIR via banded 128x128 matmuls on the PE in a chunked layout
(time n = 128*c + p: partition p, chunk column c):

    stage1 (causal):     y1[:, c] = sum_{j<J} G1_j^T @ x[:, c-j]
    mask:                y1 zeroed on [row_end, next_row_start)  (matches
                         the reference's zero-state backward truncation)
    stage2 (anticausal): y2[:, c] = sum_{j<J} G2_j^T @ y1[:, c+j]

with G1_j[p', p] = h[128j + p - p'], G2_j = G1_j^T. J=3 bands: per-partition
effective truncation at 257+p taps, ~6e-4 rel-err. bf16 storage for x/y1/y
(fp32 PSUM accumulate) halves HBM traffic, adds ~3e-3 rel-err; total
~2.9e-3 vs the fp32 reference — well under the 2e-2 gate.

The reference's per-row max-abs prescale is skipped: the pipeline is linear
in x, so scale * filt(x/scale) == filt(x) up to rounding.

Blocking: stage-1 tiles of 512 chunk-columns (one PSUM bank) advance by
STRIDE=510, so each stage-2 band (halo 2) reads entirely within one y1
tile — every matmul is full-width (N>=510) and weight loads stay hidden.
Stage 2 is emitted two blocks behind stage 1 (software pipeline) so the PE
never waits on the PSUM->SBUF copy + mask chain; the PE is the bottleneck
engine at ~1.3us/block steady state (~3072 PE cycles — its roofline).
Input/output DMAs are paired across two blocks for ~2KB per-partition
lines.

Rows are data-parallel: 40 rows per core on 8 cores, concatenated into one
chunk stream with >= NT zero gap between rows, so no communication.
"""
import math
import numpy as np
import ml_dtypes

import concourse.bass as bass
import concourse.mybir as mybir
from concourse.tile import TileContext
from concourse.vector_clock import ScopedClock
from concourse import bass_utils

# ---------------------------------------------------------------------------
# Compat patches: this walrus build supports only one sync-wait command per
# TPB_CTRL instruction, so split Tile's exit-drain waits and use the
# sem-only all-engine barrier (no eq-wait drains).
# ---------------------------------------------------------------------------
def _patched_meb(self, engines):
    for inst in self._sem_only_all_engine_barrier_insts(f"aeb{self.next_id()}"):
        self.engines[inst.engine].add_instruction(inst)


def _patched_dab(self, tick_clock, wait_clock):
    drain_inst = self.nc.sync.drain()
    wait_clock.add_sem_waits(
        drain_inst.ins, ScopedClock({None: tick_clock.global_clock})
    )
    si = drain_inst.ins.sync_info
    if si is not None and si.on_wait and len(si.on_wait) > 1:
        waits = list(si.on_wait)
        si.on_wait = waits[:1]
        for w in waits[1:]:
            d2 = self.nc.sync.drain()
            d2.ins.sync_info = mybir.SyncInfo(on_wait=[w], on_update=[])
    self.nc.all_engine_barrier()
    popped = self.nc._tile_sem_poison_stack.pop()
    assert popped is self._sem_poison
    self.nc.clear_and_free_semaphores(list(self.sems.allocated().values()))
    self.nc.all_engine_barrier()


bass.Bass.multi_engine_barrier = _patched_meb
TileContext._drain_and_barrier = _patched_dab


def _split_multi_waits(nc):
    """Walrus here allows one sync-wait command per engine instruction:
    hoist extra waits onto InstNoOp carriers inserted just before."""
    import copy as _copy
    nop_template = None
    counter = [0]

    def _mk_nop(engine, wait):
        nop = _copy.replace(nop_template, name=f"I-waitsplit-{counter[0]}")
        counter[0] += 1
        nop.engine = engine
        nop.sync_info = mybir.SyncInfo(on_wait=[wait], on_update=[])
        return nop

    m = nc.m
    for fn in m.functions:
        for blk in fn.blocks:
            need = False
            for inst in blk.instructions:
                si = inst.sync_info
                if si is not None and si.on_wait and len(si.on_wait) > 1:
                    need = True
                    break
            if not need:
                continue
            insts = []
            for inst in blk.instructions:
                si = inst.sync_info
                if si is not None and si.on_wait and len(si.on_wait) > 1:
                    if nop_template is None:
                        import bass_rust
                        nop_template = bass_rust.InstNoOp(name="I-waitsplit-t")
                    ws = list(si.on_wait)
                    for w in ws[:-1]:
                        insts.append(_mk_nop(inst.engine, w))
                    si.on_wait = ws[-1:]
                insts.append(inst)
            blk.instructions[:] = []
            for i in insts:
                blk.instructions.append(i)


# ---------------------------------------------------------------------------
# Layout constants (hardcoded for x of shape (320, 200000) on 8 cores)
# ---------------------------------------------------------------------------
T = 200000
PADLEN = 9
TXE = T + 2 * PADLEN          # 200018 odd-extended row length
NT = 384                      # truncated impulse response taps
J = 3                         # bands per stage (NT/128)
HALO = J - 1                  # chunk halo
P = 128
ROW_CHUNKS = 1568             # chunk columns per row slot
S = ROW_CHUNKS * P            # 200704 row stride in stream (gap = 686 >= NT)
P0 = 768                      # stream left pad (6 chunks)
TILE = 512                    # stage-1 tile width (one PSUM bank)
STRIDE = TILE - HALO          # 510 output cols per block
NCORES = 8
ROWS_PER_CORE = 40
DATA_CHUNKS = math.ceil((P0 + (ROWS_PER_CORE - 1) * S + TXE) / P) + 6  # 62727
NBLK = math.ceil(DATA_CHUNKS / STRIDE)       # 123
XCOLS = HALO + (NBLK - 1) * STRIDE + TILE    # 62734 input cols (incl left pad)
YCOLS = NBLK * STRIDE                        # 62730 output cols
SLAB = TILE + HALO                           # 514 input cols per block
BF16 = mybir.dt.bfloat16
F32 = mybir.dt.float32


def _impulse_response(b, a, nt):
    b = np.asarray(b, np.float64)
    a = np.asarray(a, np.float64)
    b = b / a[0]
    a = a / a[0]
    h = np.zeros(nt, np.float64)
    for n in range(nt):
        acc = b[n] if n < len(b) else 0.0
        for k in range(1, len(a)):
            if n - k >= 0:
                acc -= a[k] * h[n - k]
        h[n] = acc
    return h


def _band_matrices(h):
    idx = np.arange(P)
    G = np.zeros((2 * J, P, P), np.float64)
    for j in range(J):
        k = 128 * j + idx[None, :] - idx[:, None]     # [p', p]: k = 128j + p - p'
        valid = (k >= 0) & (k < NT)
        G[j][valid] = h[np.clip(k, 0, NT - 1)][valid]
    for j in range(J):
        G[J + j] = G[j].T
    return G.astype(np.float32)


def _mask_schedule():
    """Per block m: list of (col_lo, col_hi, p_lo, p_hi) regions of the
    block-local 512-col y1 tile to zero (gap between row end and next row
    start). Tile m covers global y1 cols [m*STRIDE, m*STRIDE+TILE)."""
    sched = {m: [] for m in range(NBLK)}
    e_rem = (P0 + TXE) % P
    for r in range(ROWS_PER_CORE):
        c_e = (P0 + r * S + TXE) // P           # partial chunk
        regions = [(c_e, c_e + 1, e_rem, P), (c_e + 1, c_e + 6, 0, P)]
        for (clo, chi, plo, phi) in regions:
            for m in range(NBLK):
                b_lo, b_hi = m * STRIDE, m * STRIDE + TILE
                lo, hi = max(clo, b_lo), min(chi, b_hi)
                if lo < hi:
                    sched[m].append((lo - b_lo, hi - b_lo, plo, phi))
    return sched


def _build(reps=1):
    nc = bass.Bass()
    g = nc.dram_tensor("g", [2 * J * P, P], BF16, kind="ExternalInput")
    mv = nc.dram_tensor("mv", [P, 1], F32, kind="ExternalInput")
    xin = nc.dram_tensor("xin", [P, XCOLS], BF16, kind="ExternalInput")
    yout = nc.dram_tensor("yout", [P, YCOLS], BF16, kind="ExternalOutput")
    sched = _mask_schedule()
    with TileContext(nc) as tc:
        with (
            tc.tile_pool(name="gp", bufs=1) as gp,
            tc.tile_pool(name="xp", bufs=3) as xp,
            tc.tile_pool(name="y1p", bufs=4) as y1p,
            tc.tile_pool(name="yp", bufs=3) as yp,
            tc.tile_pool(name="pp1", bufs=2, space="PSUM") as pp1,
            tc.tile_pool(name="pp2", bufs=2, space="PSUM") as pp2,
        ):
            gt = gp.tile([P, 2 * J * P], BF16)
            mvt = gp.tile([P, 1], F32)
            nc.sync.dma_start(mvt[:], mv[:])
            for j in range(2 * J):
                nc.sync.dma_start(gt[:, j * P:(j + 1) * P], g[j * P:(j + 1) * P, :])

            def band1(j):
                return gt[:, j * P:(j + 1) * P]

            def band2(j):
                return gt[:, (J + j) * P:(J + j + 1) * P]

            def body():
                # input DMAs fetch two blocks' slabs at once (2KB lines);
                # output DMAs flush two blocks' results at once.
                y1tiles = {}
                xtiles = {}
                ytiles = {}

                def fetch_pair(i):           # blocks 2i, 2i+1
                    lo = 2 * i * STRIDE
                    w = min(XCOLS - lo, 2 * STRIDE + HALO + HALO)
                    xt = xp.tile([P, w], BF16)
                    nc.sync.dma_start(xt[:], xin[:, lo:lo + w])
                    xtiles[2 * i] = (xt, 0)
                    if w > STRIDE + SLAB - HALO:
                        xtiles[2 * i + 1] = (xt, STRIDE)

                def stage1(k):
                    xt, off = xtiles.pop(k)
                    ps1 = pp1.tile([P, TILE], F32)
                    for j in range(J):
                        o = off + HALO - j
                        nc.tensor.matmul(ps1[:], band1(j), xt[:, o:o + TILE],
                                         start=(j == 0), stop=(j == J - 1))
                    y1t = y1p.tile([P, TILE], BF16)
                    nc.scalar.copy(y1t[:], ps1[:])
                    for (clo, chi, plo, phi) in sched[k]:
                        if plo == 0:
                            nc.gpsimd.memset(y1t[plo:phi, clo:chi], 0.0)
                        else:
                            nc.vector.tensor_scalar_mul(
                                y1t[:, clo:chi], y1t[:, clo:chi], mvt[:])
                    y1tiles[k] = y1t

                def stage2(m):
                    y1t = y1tiles.pop(m)
                    ps2 = pp2.tile([P, STRIDE], F32)
                    for j in range(J):
                        nc.tensor.matmul(ps2[:], band2(j), y1t[:, j:j + STRIDE],
                                         start=(j == 0), stop=(j == J - 1))
                    if m % 2 == 0:
                        wpair = min(YCOLS - m * STRIDE, 2 * STRIDE)
                        yt = yp.tile([P, wpair], BF16)
                        ytiles[m] = yt
                        nc.vector.tensor_copy(yt[:, :STRIDE], ps2[:])
                        if wpair == STRIDE:       # lone final even block
                            nc.sync.dma_start(
                                yout[:, m * STRIDE:m * STRIDE + STRIDE], yt[:])
                    else:
                        yt = ytiles.pop(m - 1)
                        nc.vector.tensor_copy(yt[:, STRIDE:], ps2[:])
                        nc.sync.dma_start(
                            yout[:, (m - 1) * STRIDE:(m + 1) * STRIDE], yt[:])

                npairs = math.ceil(NBLK / 2)
                for i in range(npairs):
                    fetch_pair(i)
                    for k in (2 * i, 2 * i + 1):
                        if k >= NBLK:
                            break
                        stage1(k)
                        if k >= 2:
                            stage2(k - 2)
                stage2(NBLK - 2)
                stage2(NBLK - 1)

            if reps == 1:
                body()
            else:
                with tc.For_i(0, reps, 1):
                    body()
    return nc


def _odd_ext(x):
    left = 2.0 * x[:, :1] - x[:, 1:PADLEN + 1][:, ::-1]
    right = 2.0 * x[:, -1:] - x[:, -(PADLEN + 1):-1][:, ::-1]
    return np.concatenate([left, x, right], axis=1).astype(np.float32)


FULL_CHUNKS = TXE // P        # 1562 full chunks per row
REM = TXE - FULL_CHUNKS * P   # 82 trailing samples


def _prep_core(xe_rows):
    """xe_rows: [ROWS_PER_CORE, TXE] bf16 -> xin [P, XCOLS] bf16.

    Row r occupies stream chunks [P0/P + r*ROW_CHUNKS, ...): per-row
    reshape+transpose (cache-friendly 0.4MB blocks), no giant staging
    buffer."""
    pc = np.zeros((P, XCOLS), ml_dtypes.bfloat16)
    for r in range(xe_rows.shape[0]):
        c0 = HALO + P0 // P + r * ROW_CHUNKS
        row = xe_rows[r]
        pc[:, c0:c0 + FULL_CHUNKS] = row[:FULL_CHUNKS * P].reshape(
            FULL_CHUNKS, P).T
        pc[:REM, c0 + FULL_CHUNKS] = row[FULL_CHUNKS * P:]
    return pc


def _gather_core(yblk, out_rows):
    """yblk: [P, YCOLS] bf16 -> out_rows [ROWS_PER_CORE, T] f32 (filled)."""
    nch = 1 + (T + PADLEN) // P + 1               # chunks covering one row
    for r in range(ROWS_PER_CORE):
        c0 = P0 // P + r * ROW_CHUNKS
        seq = np.ascontiguousarray(yblk[:, c0:c0 + nch].T).reshape(-1)
        out_rows[r] = seq[PADLEN:PADLEN + T].astype(np.float32)
    return out_rows


_NC_CACHE = {}


def _run(x, b, a, reps=1):
    from concurrent.futures import ThreadPoolExecutor
    x = np.asarray(x, np.float32)
    assert x.shape == (NCORES * ROWS_PER_CORE, T), x.shape
    h = _impulse_response(np.asarray(b), np.asarray(a), NT)
    g = _band_matrices(h).reshape(2 * J * P, P).astype(ml_dtypes.bfloat16)
    xe = _odd_ext(x).astype(ml_dtypes.bfloat16)
    mvec = (np.arange(P) < (P0 + TXE) % P).astype(np.float32).reshape(P, 1)
    with ThreadPoolExecutor(NCORES) as ex:
        xins = list(ex.map(
            lambda c: _prep_core(xe[c * ROWS_PER_CORE:(c + 1) * ROWS_PER_CORE]),
            range(NCORES)))
    in_maps = [{"g": g, "xin": xins[c], "mv": mvec} for c in range(NCORES)]
    if reps not in _NC_CACHE:
        nc = _build(reps)
        _split_multi_waits(nc)
        _NC_CACHE[reps] = nc
    import time
    t0 = time.perf_counter()
    res = bass_utils.run_bass_kernel_spmd(
        _NC_CACHE[reps], in_maps, core_ids=list(range(NCORES)))
    wall = time.perf_counter() - t0
    y = np.empty((NCORES * ROWS_PER_CORE, T), np.float32)
    with ThreadPoolExecutor(NCORES) as ex:
        list(ex.map(
            lambda c: _gather_core(
                res.results[c]["yout"],
                y[c * ROWS_PER_CORE:(c + 1) * ROWS_PER_CORE]),
            range(NCORES)))
    return y, wall


def kernel(x, b, a):
    y, _ = _run(x, b, a, reps=1)
    return y


# revision 6
# speedup vs baseline: 1.3310x; 1.0005x over previous
"""Trainium2 Bass kernel for nn_LowPass (order-2 Butterworth filtfilt) — v2.

filtfilt == causal IIR pass + anticausal IIR pass. Each pass is computed as a
truncated-FIR via banded 128x128 matmuls on the PE in a chunked layout
(time n = 128*c + p: partition p, chunk column c):

    stage1 (causal):     y1[:, c] = sum_{j<J} G1_j^T @ x[:, c-j]
    mask:                y1 zeroed on [row_end, next_row_start)
    stage2 (anticausal): y2[:, c] = sum_{j<J} G2_j^T @ y1[:, c+j]

with G1_j[p', p] = h[128j + p - p'], G2_j = G1_j^T. J=3 bands: per-partition
effective truncation at 257+p taps, ~6e-4 rel-err. bf16 storage for x/y1/y
(fp32 PSUM accumulate) halves HBM traffic and adds ~3e-3 rel-err.

Blocking: stage-1 tiles of 512 chunk-columns advance by STRIDE=510 so each
stage-2 band (halo 2) reads entirely within one y1 tile — every matmul is
full-width (N>=510) and weight loads stay hidden. Stage 2 is emitted two
blocks behind stage 1 (software pipeline) so the PE never waits on the
PSUM->SBUF copy + mask chain. Input/output DMAs are paired across two blocks
for ~2KB per-partition lines.

Rows are data-parallel: 40 rows per core on 8 cores, concatenated into one
chunk stream with >= NT zero gap between rows.
"""
import math
import numpy as np
import ml_dtypes

import concourse.bass as bass
import concourse.mybir as mybir
from concourse.tile import TileContext
from concourse.vector_clock import ScopedClock
from concourse import bass_utils

# ---------------------------------------------------------------------------
# Compat patches (walrus build: one sync-wait per instruction) — same as v1.
# ---------------------------------------------------------------------------
def _patched_meb(self, engines):
    for inst in self._sem_only_all_engine_barrier_insts(f"aeb{self.next_id()}"):
        self.engines[inst.engine].add_instruction(inst)


def _patched_dab(self, tick_clock, wait_clock):
    drain_inst = self.nc.sync.drain()
    wait_clock.add_sem_waits(
        drain_inst.ins, ScopedClock({None: tick_clock.global_clock})
    )
    si = drain_inst.ins.sync_info
    if si is not None and si.on_wait and len(si.on_wait) > 1:
        waits = list(si.on_wait)
        si.on_wait = waits[:1]
        for w in waits[1:]:
            d2 = self.nc.sync.drain()
            d2.ins.sync_info = mybir.SyncInfo(on_wait=[w], on_update=[])
    self.nc.all_engine_barrier()
    popped = self.nc._tile_sem_poison_stack.pop()
    assert popped is self._sem_poison
    self.nc.clear_and_free_semaphores(list(self.sems.allocated().values()))
    self.nc.all_engine_barrier()


bass.Bass.multi_engine_barrier = _patched_meb
TileContext._drain_and_barrier = _patched_dab


def _split_multi_waits(nc):
    import copy as _copy
    nop_template = None
    counter = [0]

    def _mk_nop(engine, wait):
        nop = _copy.replace(nop_template, name=f"I-waitsplit-{counter[0]}")
        counter[0] += 1
        nop.engine = engine
        nop.sync_info = mybir.SyncInfo(on_wait=[wait], on_update=[])
        return nop

    m = nc.m
    for fn in m.functions:
        for blk in fn.blocks:
            need = False
            for inst in blk.instructions:
                si = inst.sync_info
                if si is not None and si.on_wait and len(si.on_wait) > 1:
                    need = True
                    break
            if not need:
                continue
            insts = []
            for inst in blk.instructions:
                si = inst.sync_info
                if si is not None and si.on_wait and len(si.on_wait) > 1:
                    if nop_template is None:
                        import bass_rust
                        nop_template = bass_rust.InstNoOp(name="I-waitsplit-t")
                    ws = list(si.on_wait)
                    for w in ws[:-1]:
                        insts.append(_mk_nop(inst.engine, w))
                    si.on_wait = ws[-1:]
                insts.append(inst)
            blk.instructions[:] = []
            for i in insts:
                blk.instructions.append(i)


# ---------------------------------------------------------------------------
# Layout constants (hardcoded for x of shape (320, 200000) on 8 cores)
# ---------------------------------------------------------------------------
T = 200000
PADLEN = 9
TXE = T + 2 * PADLEN          # 200018 odd-extended row length
NT = 384                      # truncated impulse response taps
J = 3                         # bands per stage (NT/128)
HALO = J - 1                  # chunk halo
P = 128
ROW_CHUNKS = 1568             # chunk columns per row slot
S = ROW_CHUNKS * P            # 200704 row stride in stream (gap = 686 >= NT)
P0 = 768                      # stream left pad (6 chunks)
TILE = 512                    # stage-1 tile width (one PSUM bank)
STRIDE = TILE - HALO          # 510 output cols per block
NCORES = 8
ROWS_PER_CORE = 40
DATA_CHUNKS = math.ceil((P0 + (ROWS_PER_CORE - 1) * S + TXE) / P) + 6  # 62727
NBLK = math.ceil(DATA_CHUNKS / STRIDE)       # 123
XCOLS = HALO + (NBLK - 1) * STRIDE + TILE    # 62734 input cols (incl left pad)
YCOLS = NBLK * STRIDE                        # 62730 output cols
SLAB = TILE + HALO                           # 514 input cols per block
BF16 = mybir.dt.bfloat16
F32 = mybir.dt.float32


def _impulse_response(b, a, nt):
    b = np.asarray(b, np.float64)
    a = np.asarray(a, np.float64)
    b = b / a[0]
    a = a / a[0]
    h = np.zeros(nt, np.float64)
    for n in range(nt):
        acc = b[n] if n < len(b) else 0.0
        for k in range(1, len(a)):
            if n - k >= 0:
                acc -= a[k] * h[n - k]
        h[n] = acc
    return h


def _band_matrices(h):
    idx = np.arange(P)
    G = np.zeros((2 * J, P, P), np.float64)
    for j in range(J):
        k = 128 * j + idx[None, :] - idx[:, None]     # [p', p]: k = 128j + p - p'
        valid = (k >= 0) & (k < NT)
        G[j][valid] = h[np.clip(k, 0, NT - 1)][valid]
    for j in range(J):
        G[J + j] = G[j].T
    return G.astype(np.float32)


def _mask_schedule():
    """Per block m: list of (col_lo, col_hi, p_lo, p_hi) regions of the
    block-local 512-col y1 tile to zero (gap between row end and next row
    start). Tile m covers global y1 cols [m*STRIDE, m*STRIDE+TILE)."""
    sched = {m: [] for m in range(NBLK)}
    e_rem = (P0 + TXE) % P
    for r in range(ROWS_PER_CORE):
        c_e = (P0 + r * S + TXE) // P           # partial chunk
        regions = [(c_e, c_e + 1, e_rem, P), (c_e + 1, c_e + 6, 0, P)]
        for (clo, chi, plo, phi) in regions:
            for m in range(NBLK):
                b_lo, b_hi = m * STRIDE, m * STRIDE + TILE
                lo, hi = max(clo, b_lo), min(chi, b_hi)
                if lo < hi:
                    sched[m].append((lo - b_lo, hi - b_lo, plo, phi))
    return sched


def _build(reps=1):
    nc = bass.Bass()
    g = nc.dram_tensor("g", [2 * J * P, P], BF16, kind="ExternalInput")
    mv = nc.dram_tensor("mv", [P, 1], F32, kind="ExternalInput")
    xin = nc.dram_tensor("xin", [P, XCOLS], BF16, kind="ExternalInput")
    yout = nc.dram_tensor("yout", [P, YCOLS], BF16, kind="ExternalOutput")
    sched = _mask_schedule()
    with TileContext(nc) as tc:
        with (
            tc.tile_pool(name="gp", bufs=1) as gp,
            tc.tile_pool(name="xp", bufs=3) as xp,
            tc.tile_pool(name="y1p", bufs=4) as y1p,
            tc.tile_pool(name="yp", bufs=3) as yp,
            tc.tile_pool(name="pp1", bufs=2, space="PSUM") as pp1,
            tc.tile_pool(name="pp2", bufs=2, space="PSUM") as pp2,
        ):
            gt = gp.tile([P, 2 * J * P], BF16)
            mvt = gp.tile([P, 1], F32)
            nc.sync.dma_start(mvt[:], mv[:])
            for j in range(2 * J):
                nc.sync.dma_start(gt[:, j * P:(j + 1) * P], g[j * P:(j + 1) * P, :])

            def band1(j):
                return gt[:, j * P:(j + 1) * P]

            def band2(j):
                return gt[:, (J + j) * P:(J + j + 1) * P]

            def body():
                # input DMAs fetch two blocks' slabs at once (2KB lines);
                # output DMAs flush two blocks' results at once.
                y1tiles = {}
                xtiles = {}
                ytiles = {}

                def fetch_pair(i):           # blocks 2i, 2i+1
                    lo = 2 * i * STRIDE
                    w = min(XCOLS - lo, 2 * STRIDE + HALO + HALO)
                    xt = xp.tile([P, w], BF16)
                    nc.sync.dma_start(xt[:], xin[:, lo:lo + w])
                    xtiles[2 * i] = (xt, 0)
                    if w > STRIDE + SLAB - HALO:
                        xtiles[2 * i + 1] = (xt, STRIDE)

                def stage1(k):
                    xt, off = xtiles.pop(k)
                    ps1 = pp1.tile([P, TILE], F32)
                    for j in range(J):
                        o = off + HALO - j
                        nc.tensor.matmul(ps1[:], band1(j), xt[:, o:o + TILE],
                                         start=(j == 0), stop=(j == J - 1))
                    y1t = y1p.tile([P, TILE], BF16)
                    nc.scalar.copy(y1t[:], ps1[:])
                    for (clo, chi, plo, phi) in sched[k]:
                        if plo == 0:
                            nc.gpsimd.memset(y1t[plo:phi, clo:chi], 0.0)
                        else:
                            nc.vector.tensor_scalar_mul(
                                y1t[:, clo:chi], y1t[:, clo:chi], mvt[:])
                    y1tiles[k] = y1t

                def stage2(m):
                    y1t = y1tiles.pop(m)
                    ps2 = pp2.tile([P, STRIDE], F32)
                    for j in range(J):
                        nc.tensor.matmul(ps2[:], band2(j), y1t[:, j:j + STRIDE],
                                         start=(j == 0), stop=(j == J - 1))
                    if m % 2 == 0:
                        wpair = min(YCOLS - m * STRIDE, 2 * STRIDE)
                        yt = yp.tile([P, wpair], BF16)
                        ytiles[m] = yt
                        nc.vector.tensor_copy(yt[:, :STRIDE], ps2[:])
                        if wpair == STRIDE:       # lone final even block
                            nc.sync.dma_start(
                                yout[:, m * STRIDE:m * STRIDE + STRIDE], yt[:])
                    else:
                        yt = ytiles.pop(m - 1)
                        nc.vector.tensor_copy(yt[:, STRIDE:], ps2[:])
                        nc.sync.dma_start(
                            yout[:, (m - 1) * STRIDE:(m + 1) * STRIDE], yt[:])

                npairs = math.ceil(NBLK / 2)
                for i in range(npairs):
                    fetch_pair(i)
                    for k in (2 * i, 2 * i + 1):
                        if k >= NBLK:
                            break
                        stage1(k)
                        if k >= 2:
                            stage2(k - 2)
                stage2(NBLK - 2)
                stage2(NBLK - 1)

            if reps == 1:
                body()
            else:
                with tc.For_i(0, reps, 1):
                    body()
    return nc


def _odd_ext(x):
    left = 2.0 * x[:, :1] - x[:, 1:PADLEN + 1][:, ::-1]
    right = 2.0 * x[:, -1:] - x[:, -(PADLEN + 1):-1][:, ::-1]
    return np.concatenate([left, x, right], axis=1).astype(np.float32)


FULL_CHUNKS = TXE // P        # 1562 full chunks per row
REM = TXE - FULL_CHUNKS * P   # 82 trailing samples


def _prep_core(xe_rows):
    """xe_rows: [ROWS_PER_CORE, TXE] bf16 -> xin [P, XCOLS] bf16."""
    pc = np.zeros((P, XCOLS), ml_dtypes.bfloat16)
    for r in range(xe_rows.shape[0]):
        c0 = HALO + P0 // P + r * ROW_CHUNKS
        row = xe_rows[r]
        pc[:, c0:c0 + FULL_CHUNKS] = row[:FULL_CHUNKS * P].reshape(
            FULL_CHUNKS, P).T
        pc[:REM, c0 + FULL_CHUNKS] = row[FULL_CHUNKS * P:]
    return pc


def _gather_core(yblk, out_rows):
    """yblk: [P, YCOLS] bf16 -> out_rows [ROWS_PER_CORE, T] f32 (filled)."""
    nch = 1 + (T + PADLEN) // P + 1               # chunks covering one row
    for r in range(ROWS_PER_CORE):
        c0 = P0 // P + r * ROW_CHUNKS
        seq = np.ascontiguousarray(yblk[:, c0:c0 + nch].T).reshape(-1)
        out_rows[r] = seq[PADLEN:PADLEN + T].astype(np.float32)
    return out_rows


_NC_CACHE = {}


def _run(x, b, a, reps=1):
    from concurrent.futures import ThreadPoolExecutor
    x = np.asarray(x, np.float32)
    assert x.shape == (NCORES * ROWS_PER_CORE, T), x.shape
    h = _impulse_response(np.asarray(b), np.asarray(a), NT)
    g = _band_matrices(h).reshape(2 * J * P, P).astype(ml_dtypes.bfloat16)
    xe = _odd_ext(x).astype(ml_dtypes.bfloat16)
    mvec = (np.arange(P) < (P0 + TXE) % P).astype(np.float32).reshape(P, 1)
    with ThreadPoolExecutor(NCORES) as ex:
        xins = list(ex.map(
            lambda c: _prep_core(xe[c * ROWS_PER_CORE:(c + 1) * ROWS_PER_CORE]),
            range(NCORES)))
    in_maps = [{"g": g, "xin": xins[c], "mv": mvec} for c in range(NCORES)]
    if reps not in _NC_CACHE:
        nc = _build(reps)
        _split_multi_waits(nc)
        _NC_CACHE[reps] = nc
    import time
    t0 = time.perf_counter()
    res = bass_utils.run_bass_kernel_spmd(
        _NC_CACHE[reps], in_maps, core_ids=list(range(NCORES)))
    wall = time.perf_counter() - t0
    y = np.empty((NCORES * ROWS_PER_CORE, T), np.float32)
    with ThreadPoolExecutor(NCORES) as ex:
        list(ex.map(
            lambda c: _gather_core(
                res.results[c]["yout"],
                y[c * ROWS_PER_CORE:(c + 1) * ROWS_PER_CORE]),
            range(NCORES)))
    return y, wall


def kernel(x, b, a):
    y, _ = _run(x, b, a, reps=1)
    return y


# revision 7
# speedup vs baseline: 1.3767x; 1.0343x over previous
"""Trainium2 Bass kernel for nn_LowPass (order-2 Butterworth filtfilt) — v2.

filtfilt == causal IIR pass + anticausal IIR pass. Each pass is computed as a
truncated-FIR via banded 128x128 matmuls on the PE in a chunked layout
(time n = 128*c + p: partition p, chunk column c):

    stage1 (causal):     y1[:, c] = sum_{j<J} G1_j^T @ x[:, c-j]
    mask:                y1 zeroed on [row_end, next_row_start)
    stage2 (anticausal): y2[:, c] = sum_{j<J} G2_j^T @ y1[:, c+j]

with G1_j[p', p] = h[128j + p - p'], G2_j = G1_j^T. J=3 bands: per-partition
effective truncation at 257+p taps, ~6e-4 rel-err. bf16 storage for x/y1/y
(fp32 PSUM accumulate) halves HBM traffic and adds ~3e-3 rel-err.

Blocking: stage-1 tiles of 512 chunk-columns advance by STRIDE=510 so each
stage-2 band (halo 2) reads entirely within one y1 tile — every matmul is
full-width (N>=510) and weight loads stay hidden. Stage 2 is emitted two
blocks behind stage 1 (software pipeline) so the PE never waits on the
PSUM->SBUF copy + mask chain. Input/output DMAs are paired across two blocks
for ~2KB per-partition lines.

Rows are data-parallel: 40 rows per core on 8 cores, concatenated into one
chunk stream with >= NT zero gap between rows.
"""
import math
import numpy as np
import ml_dtypes

import concourse.bass as bass
import concourse.mybir as mybir
from concourse.tile import TileContext
from concourse.vector_clock import ScopedClock
from concourse import bass_utils

# ---------------------------------------------------------------------------
# Compat patches (walrus build: one sync-wait per instruction) — same as v1.
# ---------------------------------------------------------------------------
def _patched_meb(self, engines):
    for inst in self._sem_only_all_engine_barrier_insts(f"aeb{self.next_id()}"):
        self.engines[inst.engine].add_instruction(inst)


def _patched_dab(self, tick_clock, wait_clock):
    drain_inst = self.nc.sync.drain()
    wait_clock.add_sem_waits(
        drain_inst.ins, ScopedClock({None: tick_clock.global_clock})
    )
    si = drain_inst.ins.sync_info
    if si is not None and si.on_wait and len(si.on_wait) > 1:
        waits = list(si.on_wait)
        si.on_wait = waits[:1]
        for w in waits[1:]:
            d2 = self.nc.sync.drain()
            d2.ins.sync_info = mybir.SyncInfo(on_wait=[w], on_update=[])
    self.nc.all_engine_barrier()
    popped = self.nc._tile_sem_poison_stack.pop()
    assert popped is self._sem_poison
    self.nc.clear_and_free_semaphores(list(self.sems.allocated().values()))
    self.nc.all_engine_barrier()


bass.Bass.multi_engine_barrier = _patched_meb
TileContext._drain_and_barrier = _patched_dab


def _split_multi_waits(nc):
    import copy as _copy
    nop_template = None
    counter = [0]

    def _mk_nop(engine, wait):
        nop = _copy.replace(nop_template, name=f"I-waitsplit-{counter[0]}")
        counter[0] += 1
        nop.engine = engine
        nop.sync_info = mybir.SyncInfo(on_wait=[wait], on_update=[])
        return nop

    m = nc.m
    for fn in m.functions:
        for blk in fn.blocks:
            need = False
            for inst in blk.instructions:
                si = inst.sync_info
                if si is not None and si.on_wait and len(si.on_wait) > 1:
                    need = True
                    break
            if not need:
                continue
            insts = []
            for inst in blk.instructions:
                si = inst.sync_info
                if si is not None and si.on_wait and len(si.on_wait) > 1:
                    if nop_template is None:
                        import bass_rust
                        nop_template = bass_rust.InstNoOp(name="I-waitsplit-t")
                    ws = list(si.on_wait)
                    for w in ws[:-1]:
                        insts.append(_mk_nop(inst.engine, w))
                    si.on_wait = ws[-1:]
                insts.append(inst)
            blk.instructions[:] = []
            for i in insts:
                blk.instructions.append(i)


# ---------------------------------------------------------------------------
# Layout constants (hardcoded for x of shape (320, 200000) on 8 cores)
# ---------------------------------------------------------------------------
T = 200000
PADLEN = 9
TXE = T + 2 * PADLEN          # 200018 odd-extended row length
NT = 384                      # truncated impulse response taps
J = 3                         # bands per stage (NT/128)
HALO = J - 1                  # chunk halo
P = 128
ROW_CHUNKS = 1568             # chunk columns per row slot
S = ROW_CHUNKS * P            # 200704 row stride in stream (gap = 686 >= NT)
P0 = 768                      # stream left pad (6 chunks)
TILE = 512                    # stage-1 tile width (one PSUM bank)
STRIDE = TILE - HALO          # 510 output cols per block
NCORES = 8
ROWS_PER_CORE = 40
DATA_CHUNKS = math.ceil((P0 + (ROWS_PER_CORE - 1) * S + TXE) / P) + 6  # 62727
NBLK = math.ceil(DATA_CHUNKS / STRIDE)       # 123
XCOLS = HALO + (NBLK - 1) * STRIDE + TILE    # 62734 input cols (incl left pad)
YCOLS = NBLK * STRIDE                        # 62730 output cols
SLAB = TILE + HALO                           # 514 input cols per block
BF16 = mybir.dt.bfloat16
F32 = mybir.dt.float32


def _impulse_response(b, a, nt):
    b = np.asarray(b, np.float64)
    a = np.asarray(a, np.float64)
    b = b / a[0]
    a = a / a[0]
    h = np.zeros(nt, np.float64)
    for n in range(nt):
        acc = b[n] if n < len(b) else 0.0
        for k in range(1, len(a)):
            if n - k >= 0:
                acc -= a[k] * h[n - k]
        h[n] = acc
    return h


def _band_matrices(h):
    idx = np.arange(P)
    G = np.zeros((2 * J, P, P), np.float64)
    for j in range(J):
        k = 128 * j + idx[None, :] - idx[:, None]     # [p', p]: k = 128j + p - p'
        valid = (k >= 0) & (k < NT)
        G[j][valid] = h[np.clip(k, 0, NT - 1)][valid]
    for j in range(J):
        G[J + j] = G[j].T
    return G.astype(np.float32)


def _mask_schedule():
    """Per block m: list of (col_lo, col_hi, p_lo, p_hi) regions of the
    block-local 512-col y1 tile to zero (gap between row end and next row
    start). Tile m covers global y1 cols [m*STRIDE, m*STRIDE+TILE)."""
    sched = {m: [] for m in range(NBLK)}
    e_rem = (P0 + TXE) % P
    for r in range(ROWS_PER_CORE):
        c_e = (P0 + r * S + TXE) // P           # partial chunk
        regions = [(c_e, c_e + 1, e_rem, P), (c_e + 1, c_e + 6, 0, P)]
        for (clo, chi, plo, phi) in regions:
            for m in range(NBLK):
                b_lo, b_hi = m * STRIDE, m * STRIDE + TILE
                lo, hi = max(clo, b_lo), min(chi, b_hi)
                if lo < hi:
                    sched[m].append((lo - b_lo, hi - b_lo, plo, phi))
    return sched


def _build(reps=1):
    nc = bass.Bass()
    g = nc.dram_tensor("g", [2 * J * P, P], BF16, kind="ExternalInput")
    mv = nc.dram_tensor("mv", [P, 1], F32, kind="ExternalInput")
    xin = nc.dram_tensor("xin", [P, XCOLS], BF16, kind="ExternalInput")
    yout = nc.dram_tensor("yout", [P, YCOLS], BF16, kind="ExternalOutput")
    sched = _mask_schedule()
    with TileContext(nc) as tc:
        with (
            tc.tile_pool(name="gp", bufs=1) as gp,
            tc.tile_pool(name="xp", bufs=3) as xp,
            tc.tile_pool(name="y1p", bufs=4) as y1p,
            tc.tile_pool(name="yp", bufs=3) as yp,
            tc.tile_pool(name="pp1", bufs=2, space="PSUM") as pp1,
            tc.tile_pool(name="pp2", bufs=2, space="PSUM") as pp2,
        ):
            gt = gp.tile([P, 2 * J * P], BF16)
            mvt = gp.tile([P, 1], F32)
            nc.sync.dma_start(mvt[:], mv[:])
            for j in range(2 * J):
                nc.sync.dma_start(gt[:, j * P:(j + 1) * P], g[j * P:(j + 1) * P, :])

            def band1(j):
                return gt[:, j * P:(j + 1) * P]

            def band2(j):
                return gt[:, (J + j) * P:(J + j + 1) * P]

            def body():
                # input DMAs fetch two blocks' slabs at once (2KB lines);
                # output DMAs flush two blocks' results at once. The two
                # blocks of a pair are interleaved band-by-band so
                # consecutive matmuls reuse the loaded weights and alternate
                # PSUM banks (pipelines better than serial 3-MM chains).
                y1tiles = {}
                xtiles = {}
                ytiles = {}

                def fetch_pair(i):           # blocks 2i, 2i+1
                    lo = 2 * i * STRIDE
                    w = min(XCOLS - lo, 2 * STRIDE + HALO + HALO)
                    xt = xp.tile([P, w], BF16)
                    nc.sync.dma_start(xt[:], xin[:, lo:lo + w])
                    xtiles[2 * i] = (xt, 0)
                    if w > STRIDE + SLAB - HALO:
                        xtiles[2 * i + 1] = (xt, STRIDE)

                def finish_y1(k, ps1):
                    y1t = y1p.tile([P, TILE], BF16)
                    nc.scalar.copy(y1t[:], ps1[:])
                    for (clo, chi, plo, phi) in sched[k]:
                        if plo == 0:
                            nc.gpsimd.memset(y1t[plo:phi, clo:chi], 0.0)
                        else:
                            nc.vector.tensor_scalar_mul(
                                y1t[:, clo:chi], y1t[:, clo:chi], mvt[:])
                    y1tiles[k] = y1t

                def stage1_pair(ka, kb):
                    xta, offa = xtiles.pop(ka)
                    psa = pp1.tile([P, TILE], F32)
                    if kb is not None:
                        xtb, offb = xtiles.pop(kb)
                        psb = pp1.tile([P, TILE], F32)
                    for j in range(J):
                        oa = offa + HALO - j
                        nc.tensor.matmul(psa[:], band1(j), xta[:, oa:oa + TILE],
                                         start=(j == 0), stop=(j == J - 1),
                                         skip_group_check=True)
                        if kb is not None:
                            ob = offb + HALO - j
                            nc.tensor.matmul(psb[:], band1(j),
                                             xtb[:, ob:ob + TILE],
                                             start=(j == 0), stop=(j == J - 1),
                                             skip_group_check=True)
                    finish_y1(ka, psa)
                    if kb is not None:
                        finish_y1(kb, psb)

                def stage2_pair(ma, mb):
                    y1a = y1tiles.pop(ma)
                    psa = pp2.tile([P, STRIDE], F32)
                    if mb is not None:
                        y1b = y1tiles.pop(mb)
                        psb = pp2.tile([P, STRIDE], F32)
                    for j in range(J):
                        nc.tensor.matmul(psa[:], band2(j),
                                         y1a[:, j:j + STRIDE],
                                         start=(j == 0), stop=(j == J - 1),
                                         skip_group_check=True)
                        if mb is not None:
                            nc.tensor.matmul(psb[:], band2(j),
                                             y1b[:, j:j + STRIDE],
                                             start=(j == 0), stop=(j == J - 1),
                                             skip_group_check=True)
                    wpair = min(YCOLS - ma * STRIDE, 2 * STRIDE)
                    yt = yp.tile([P, wpair], BF16)
                    nc.vector.tensor_copy(yt[:, :STRIDE], psa[:])
                    if mb is not None:
                        nc.vector.tensor_copy(yt[:, STRIDE:], psb[:])
                    nc.sync.dma_start(
                        yout[:, ma * STRIDE:ma * STRIDE + wpair], yt[:])

                npairs = math.ceil(NBLK / 2)
                for i in range(npairs):
                    fetch_pair(i)
                    ka, kb = 2 * i, 2 * i + 1
                    stage1_pair(ka, kb if kb < NBLK else None)
                    if i >= 1:
                        ma = 2 * (i - 1)
                        stage1_done = ma + 1 if ma + 1 < NBLK else None
                        stage2_pair(ma, stage1_done)
                ma = 2 * (npairs - 1)
                stage2_pair(ma, ma + 1 if ma + 1 < NBLK else None)

            if reps == 1:
                body()
            else:
                with tc.For_i(0, reps, 1):
                    body()
    return nc


def _odd_ext(x):
    left = 2.0 * x[:, :1] - x[:, 1:PADLEN + 1][:, ::-1]
    right = 2.0 * x[:, -1:] - x[:, -(PADLEN + 1):-1][:, ::-1]
    return np.concatenate([left, x, right], axis=1).astype(np.float32)


FULL_CHUNKS = TXE // P        # 1562 full chunks per row
REM = TXE - FULL_CHUNKS * P   # 82 trailing samples


def _prep_core(xe_rows):
    """xe_rows: [ROWS_PER_CORE, TXE] bf16 -> xin [P, XCOLS] bf16."""
    pc = np.zeros((P, XCOLS), ml_dtypes.bfloat16)
    for r in range(xe_rows.shape[0]):
        c0 = HALO + P0 // P + r * ROW_CHUNKS
        row = xe_rows[r]
        pc[:, c0:c0 + FULL_CHUNKS] = row[:FULL_CHUNKS * P].reshape(
            FULL_CHUNKS, P).T
        pc[:REM, c0 + FULL_CHUNKS] = row[FULL_CHUNKS * P:]
    return pc


def _gather_core(yblk, out_rows):
    """yblk: [P, YCOLS] bf16 -> out_rows [ROWS_PER_CORE, T] f32 (filled)."""
    nch = 1 + (T + PADLEN) // P + 1               # chunks covering one row
    for r in range(ROWS_PER_CORE):
        c0 = P0 // P + r * ROW_CHUNKS
        seq = np.ascontiguousarray(yblk[:, c0:c0 + nch].T).reshape(-1)
        out_rows[r] = seq[PADLEN:PADLEN + T].astype(np.float32)
    return out_rows


_NC_CACHE = {}


def _run(x, b, a, reps=1):
    from concurrent.futures import ThreadPoolExecutor
    x = np.asarray(x, np.float32)
    assert x.shape == (NCORES * ROWS_PER_CORE, T), x.shape
    h = _impulse_response(np.asarray(b), np.asarray(a), NT)
    g = _band_matrices(h).reshape(2 * J * P, P).astype(ml_dtypes.bfloat16)
    xe = _odd_ext(x).astype(ml_dtypes.bfloat16)
    mvec = (np.arange(P) < (P0 + TXE) % P).astype(np.float32).reshape(P, 1)
    with ThreadPoolExecutor(NCORES) as ex:
        xins = list(ex.map(
            lambda c: _prep_core(xe[c * ROWS_PER_CORE:(c + 1) * ROWS_PER_CORE]),
            range(NCORES)))
    in_maps = [{"g": g, "xin": xins[c], "mv": mvec} for c in range(NCORES)]
    if reps not in _NC_CACHE:
        nc = _build(reps)
        _split_multi_waits(nc)
        _NC_CACHE[reps] = nc
    import time
    t0 = time.perf_counter()
    res = bass_utils.run_bass_kernel_spmd(
        _NC_CACHE[reps], in_maps, core_ids=list(range(NCORES)))
    wall = time.perf_counter() - t0
    y = np.empty((NCORES * ROWS_PER_CORE, T), np.float32)
    with ThreadPoolExecutor(NCORES) as ex:
        list(ex.map(
            lambda c: _gather_core(
                res.results[c]["yout"],
                y[c * ROWS_PER_CORE:(c + 1) * ROWS_PER_CORE]),
            range(NCORES)))
    return y, wall


def kernel(x, b, a):
    y, _ = _run(x, b, a, reps=1)
    return y
